# revision 1
# baseline (speedup 1.0000x reference)
"""GCN message-passing layer (copy_src -> segment_sum -> dual degree norm)
on 8 Trainium2 NeuronCores.

Strategy (dst-sharded message passing):
  Host side (sharding/metadata only):
    - node_f = concat(u_f, v_f); degree norms folded into one per-edge
      weight w[e] = out_norm[src[e]] * in_norm[dst[e]].
    - Edges bucketed by (core = dst range of 12500, block = 256-dst tile,
      window = 25000-src range so gather indices fit int16), padded to
      128-edge chunks with -1 indices (skipped by the DMA engine).
  Device side (per core, one static SPMD program):
    - gpsimd: dma_gather of the 512B source-feature rows, one call per
      (block, window) bucket.
    - DVE: weighted one-hot S[e, slot] = w[e] * (slot == dst_local[e]),
      built in a single tensor_scalar(is_equal, mult) per chunk.
    - PE: psum[feat(128), slot(256)] += M[e, feat].T @ S[e, slot] in
      float32r (1 cycle/row at N=256), accumulated over a block's chunks.
    - ACT: psum -> SBUF eviction per block; SP: output DMA.
  Host: transpose/concat the per-core [128 feat, 12544 slot] outputs.
"""

import math
from contextlib import ExitStack
from dataclasses import dataclass, field

import numpy as np

P = 128  # SBUF partitions / chunk size (edges per matmul)


def cdiv(a, b):
    return -(-a // b)


@dataclass(frozen=True)
class Cfg:
    n_nodes: int = 100000
    d: int = 128
    n_cores: int = 8
    blk: int = 256      # dst nodes per psum block (matmul N dim)
    win: int = 20000    # src window rows (must be < 32768 for int16 idxs)
    cpb: int = 8        # chunks per (block, window) bucket (set from data)
    nb_m: int = 4       # gather-destination (M tile) buffers
    nb_s: int = 8       # one-hot (S tile) buffers

    @property
    def dpc(self):  # dst nodes per core
        return self.n_nodes // self.n_cores

    @property
    def nblk(self):  # blocks per core
        return cdiv(self.dpc, self.blk)

    @property
    def n_win(self):
        return cdiv(self.n_nodes, self.win)

    @property
    def ncalls(self):  # gather calls per core (= buckets per core)
        return self.nblk * self.n_win

    @property
    def nchunks(self):
        return self.ncalls * self.cpb

    @property
    def spc(self):  # padded idx slots per call
        return self.cpb * P


def prep_host(u_f, v_f, src, dst, cfg: Cfg | None = None):
    """Bucket/pad edges; returns (cfg, per-core input maps)."""
    u_f = np.asarray(u_f, dtype=np.float32)
    v_f = np.asarray(v_f, dtype=np.float32)
    src = np.asarray(src).astype(np.int64)
    dst = np.asarray(dst).astype(np.int64)
    base = cfg or Cfg()
    N, NC = base.n_nodes, base.n_cores
    E = src.shape[0]

    node_f = np.ascontiguousarray(np.concatenate([u_f, v_f], axis=0))
    assert node_f.shape == (N, base.d)
    # Pre-round features to fp32r (what the PE's fp32r path computes with
    # anyway) so the gather feeds properly-rounded fp32r matmul operands.
    import neuron_dtypes

    node_f = np.ascontiguousarray(
        neuron_dtypes.static_cast_fp32_to_fp32r(node_f).view(np.float32)
    ).reshape(N, base.d)

    deg_out = np.bincount(src, minlength=N).astype(np.float32)
    deg_in = np.bincount(dst, minlength=N).astype(np.float32)
    out_norm = np.power(np.clip(deg_out, 1.0, None), np.float32(-0.5))
    in_norm = np.power(np.clip(deg_in, 1.0, None), np.float32(-0.5))
    w_edge = (out_norm[src] * in_norm[dst]).astype(np.float32)

    core = dst // base.dpc
    dst_loc = dst % base.dpc
    blk_id = dst_loc // base.blk
    slot = (dst_loc % base.blk).astype(np.float32)
    win_id = src // base.win
    idx16 = (src % base.win).astype(np.int16)

    nblk, W = base.nblk, base.n_win
    bucket = (core * nblk + blk_id) * W + win_id
    nbuckets = NC * nblk * W
    counts = np.bincount(bucket, minlength=nbuckets)
    # The SWDGE descriptor ring holds ~1024 descriptors; one gather call
    # per bucket requires every bucket to stay under that.
    assert counts.max() <= 1024, (
        f"bucket overflow: {counts.max()} edges > 1024; reduce cfg.win"
    )
    cpb = max(1, cdiv(int(counts.max()), P))
    cfg = Cfg(
        n_nodes=base.n_nodes, d=base.d, n_cores=base.n_cores, blk=base.blk,
        win=base.win, cpb=cpb, nb_m=base.nb_m, nb_s=base.nb_s,
    )
    S = cfg.spc

    order = np.argsort(bucket, kind="stable")
    starts = np.zeros(nbuckets + 1, np.int64)
    np.cumsum(counts, out=starts[1:])
    offs = np.arange(E, dtype=np.int64) - starts[bucket[order]]
    pos = bucket[order] * S + offs

    idx_stream = np.full(nbuckets * S, -1, np.int16)
    slot_stream = np.full(nbuckets * S, -1.0, np.float32)
    w_stream = np.zeros(nbuckets * S, np.float32)
    idx_stream[pos] = idx16[order]
    slot_stream[pos] = slot[order]
    w_stream[pos] = w_edge[order]

    cnts = counts.astype(np.int32)
    empty = cnts == 0
    if empty.any():
        # dma_gather needs >= 1 valid idx per call; gather row 0 with w=0.
        idx_stream[np.nonzero(empty)[0] * S] = 0
        cnts[empty] = 1

    per_core = cfg.ncalls * S
    in_maps = []
    for c in range(NC):
        seg = slice(c * per_core, (c + 1) * per_core)
        xi = idx_stream[seg].reshape(cfg.ncalls, S // 16, 16)
        xi = np.ascontiguousarray(
            np.tile(xi.transpose(2, 0, 1).reshape(16, -1), (8, 1))
        )
        sl = np.ascontiguousarray(slot_stream[seg].reshape(-1, P).T)
        wv = np.ascontiguousarray(w_stream[seg].reshape(-1, P).T)
        cn = np.ascontiguousarray(
            cnts[c * cfg.ncalls:(c + 1) * cfg.ncalls].reshape(1, -1)
        )
        in_maps.append(
            {"nf": node_f, "idx": xi, "slots": sl, "wvals": wv, "ncnt": cn}
        )
    return cfg, in_maps


def build_nc(cfg: Cfg):
    import concourse.bacc as bacc
    import concourse.mybir as mybir
    from concourse.library_config import mlp

    f32 = mybir.dt.float32
    f32r = mybir.dt.float32r
    D, W, cpb, nblk = cfg.d, cfg.n_win, cfg.cpb, cfg.nblk
    ncalls, nchunks = cfg.ncalls, cfg.nchunks
    idx_cols = ncalls * cfg.spc // 16

    nc = bacc.Bacc("TRN2", target_bir_lowering=False)

    nf = nc.dram_tensor("nf", [cfg.n_nodes, D], f32r, kind="ExternalInput")
    idx_d = nc.dram_tensor("idx", [P, idx_cols], mybir.dt.int16, kind="ExternalInput")
    slots_d = nc.dram_tensor("slots", [P, nchunks], f32, kind="ExternalInput")
    wv_d = nc.dram_tensor("wvals", [P, nchunks], f32, kind="ExternalInput")
    cnt_d = nc.dram_tensor("ncnt", [1, ncalls], mybir.dt.int32, kind="ExternalInput")
    out_d = nc.dram_tensor("out", [P, nblk * cfg.blk], f32, kind="ExternalOutput")

    with ExitStack() as ctx:
        ec = ctx.enter_context
        idx_sb = ec(nc.sbuf_tensor("idx_sb", [P, idx_cols], mybir.dt.int16))
        slots_sb = ec(nc.sbuf_tensor("slots_sb", [P, nchunks], f32))
        wv_sb = ec(nc.sbuf_tensor("wv_sb", [P, nchunks], f32))
        cnt_sb = ec(nc.sbuf_tensor("cnt_sb", [1, ncalls], mybir.dt.int32))
        iota_sb = ec(nc.sbuf_tensor("iota_sb", [P, cfg.blk], f32))
        m_sbs = [ec(nc.sbuf_tensor(f"m{j}", [P, cpb, D], f32r)) for j in range(cfg.nb_m)]
        s_sbs = [ec(nc.sbuf_tensor(f"s{j}", [P, cfg.blk], f32r)) for j in range(cfg.nb_s)]
        obufs = [ec(nc.sbuf_tensor(f"ob{j}", [P, cfg.blk], f32)) for j in range(2)]
        psums = [ec(nc.psum_tensor(f"ps{j}", [P, cfg.blk], f32)) for j in range(2)]

        io = ec(nc.semaphore("io"))
        init = ec(nc.semaphore("init"))
        gsems = [ec(nc.semaphore(f"gat{j}")) for j in range(cfg.nb_m)]
        sdve = ec(nc.semaphore("sdve"))
        pe = ec(nc.semaphore("pe"))
        ev = ec(nc.semaphore("ev"))
        osems = [ec(nc.semaphore(f"odma{j}")) for j in range(2)]

        with nc.Block() as block:

            @block.sync
            def _(sync):
                sync.dma_start(idx_sb[:], idx_d[:]).then_inc(io, 16)
                sync.dma_start(slots_sb[:], slots_d[:]).then_inc(io, 16)
                sync.dma_start(wv_sb[:], wv_d[:]).then_inc(io, 16)
                sync.dma_start(cnt_sb[:], cnt_d[:]).then_inc(io, 16)
                for b in range(nblk):
                    sync.wait_ge(ev, b + 1)
                    sync.dma_start(
                        out_d[:, b * cfg.blk:(b + 1) * cfg.blk], obufs[b % 2][:]
                    ).then_inc(osems[b % 2], 16)
                sync.wait_ge(osems[0], 16 * cdiv(nblk, 2))
                if nblk > 1:
                    sync.wait_ge(osems[1], 16 * (nblk // 2))

            @block.gpsimd
            def _(g):
                g.iota(
                    iota_sb[:], [[1, cfg.blk]], channel_multiplier=0,
                    allow_small_or_imprecise_dtypes=True,
                ).then_inc(init, 1)
                for j in range(cfg.nb_m):
                    g.memset(m_sbs[j][:].bitcast(f32), 0).then_inc(init, 1)
                g.load_library(mlp)
                g.wait_ge(init, 1 + cfg.nb_m)
                g.wait_ge(io, 64)
                with g.register("cnt") as cnt:
                    for k in range(ncalls):
                        w = k % W
                        if k >= cfg.nb_m:
                            g.wait_ge(pe, (k - cfg.nb_m + 1) * cpb)
                        g.reg_load(cnt, cnt_sb[0:1, k:k + 1])
                        rows = min(cfg.win, cfg.n_nodes - w * cfg.win)
                        g.dma_gather(
                            m_sbs[k % cfg.nb_m][:],
                            nf[w * cfg.win: w * cfg.win + rows, :],
                            idx_sb[:, k * cpb * 8:(k + 1) * cpb * 8],
                            cfg.spc,
                            cnt,
                            D,
                        ).then_inc(gsems[k % cfg.nb_m], 16)

            @block.vector
            def _(v):
                v.wait_ge(io, 64)
                v.wait_ge(init, 1)
                for t in range(nchunks):
                    if t >= cfg.nb_s:
                        v.wait_ge(pe, t - cfg.nb_s + 1)
                    v.tensor_scalar(
                        out=s_sbs[t % cfg.nb_s][:],
                        in0=iota_sb[:],
                        scalar1=slots_sb[:, t:t + 1],
                        scalar2=wv_sb[:, t:t + 1],
                        op0=mybir.AluOpType.is_equal,
                        op1=mybir.AluOpType.mult,
                    ).then_inc(sdve, 1)

            @block.tensor
            def _(te):
                t = 0
                for b in range(nblk):
                    for w in range(W):
                        k = b * W + w
                        for i in range(cpb):
                            if i == 0:
                                te.wait_ge(
                                    gsems[k % cfg.nb_m],
                                    16 * (k // cfg.nb_m + 1),
                                )
                            te.wait_ge(sdve, t + 1)
                            start = (w == 0 and i == 0)
                            stop = (w == W - 1 and i == cpb - 1)
                            if start and b >= 2:
                                te.wait_ge(ev, b - 1)
                            te.matmul(
                                psums[b % 2][:],
                                m_sbs[k % cfg.nb_m][:, i, :],
                                s_sbs[t % cfg.nb_s][:],
                                start=start,
                                stop=stop,
                            ).then_inc(pe, 1)
                            t += 1

            @block.scalar
            def _(a):
                for b in range(nblk):
                    a.wait_ge(pe, (b + 1) * W * cpb)
                    if b >= 2:
                        a.wait_ge(osems[b % 2], 16 * (b // 2))
                    a.activation(
                        obufs[b % 2][:], psums[b % 2][:],
                        mybir.ActivationFunctionType.Copy,
                    ).then_inc(ev, 1)

    nc.compile()
    return nc


def unshard(cfg: Cfg, results):
    out = np.empty((cfg.n_nodes, cfg.d), np.float32)
    for c in range(cfg.n_cores):
        o = results[c]["out"]
        out[c * cfg.dpc:(c + 1) * cfg.dpc, :] = o[:, :cfg.dpc].T
    return out


def run(inputs, trace=False, **spmd_kwargs):
    from concourse.bass_utils import run_bass_kernel_spmd

    cfg, in_maps = prep_host(
        inputs["u_f"], inputs["v_f"], inputs["src"], inputs["dst"]
    )
    nc = build_nc(cfg)
    res = run_bass_kernel_spmd(
        nc, in_maps, core_ids=list(range(cfg.n_cores)), trace=trace,
        **spmd_kwargs,
    )
    return unshard(cfg, res.results), res


def kernel(**inputs):
    return run(inputs)[0]



# revision 2
# speedup vs baseline: 1.1156x; 1.1156x over previous
"""GCN message-passing layer on 8 Trainium2 NeuronCores — v3.

vs baseline:
  - 4 SWDGE queues, gather calls round-robin across Q7 CPU pairs.
  - bf16 node features (out_norm[src] pre-folded on host) and bf16 one-hot.
  - S built with ONE tensor_tensor is_equal against a stride-0 broadcast
    of the slot id (no per-partition scalar-AP loads).
  - in_norm[dst] folded into the PSUM eviction (tensor_tensor mult by a
    partition-broadcast in_norm row) on DVE; ACT unused.
"""

import math
from contextlib import ExitStack
from dataclasses import dataclass, field

import numpy as np

P = 128  # SBUF partitions / chunk size (edges per matmul)


def cdiv(a, b):
    return -(-a // b)


@dataclass(frozen=True)
class Cfg:
    n_nodes: int = 100000
    d: int = 128
    n_cores: int = 8
    blk: int = 256      # dst nodes per psum block (matmul N dim)
    win: int = 20000    # src window rows (must be < 32768 for int16 idxs)
    cpb: int = 8        # chunks per (block, window) bucket (set from data)
    nb_m: int = 6       # gather-destination (M tile) buffers
    nb_s: int = 8       # one-hot (S tile) buffers

    @property
    def dpc(self):  # dst nodes per core
        return self.n_nodes // self.n_cores

    @property
    def nblk(self):  # blocks per core
        return cdiv(self.dpc, self.blk)

    @property
    def n_win(self):
        return cdiv(self.n_nodes, self.win)

    @property
    def ncalls(self):  # gather calls per core (= buckets per core)
        return self.nblk * self.n_win

    @property
    def nchunks(self):
        return self.ncalls * self.cpb

    @property
    def spc(self):  # padded idx slots per call
        return self.cpb * P


def prep_host(u_f, v_f, src, dst, cfg: Cfg | None = None):
    """Bucket/pad edges; returns (cfg, per-core input maps)."""
    import ml_dtypes

    u_f = np.asarray(u_f, dtype=np.float32)
    v_f = np.asarray(v_f, dtype=np.float32)
    src = np.asarray(src).astype(np.int64)
    dst = np.asarray(dst).astype(np.int64)
    base = cfg or Cfg()
    N, NC = base.n_nodes, base.n_cores
    E = src.shape[0]

    deg_out = np.bincount(src, minlength=N).astype(np.float32)
    deg_in = np.bincount(dst, minlength=N).astype(np.float32)
    out_norm = np.power(np.clip(deg_out, 1.0, None), np.float32(-0.5))
    in_norm = np.power(np.clip(deg_in, 1.0, None), np.float32(-0.5))

    node_f = np.concatenate([u_f, v_f], axis=0) * out_norm[:, None]
    node_f = np.ascontiguousarray(node_f.astype(ml_dtypes.bfloat16))
    assert node_f.shape == (N, base.d)

    core = dst // base.dpc
    dst_loc = dst % base.dpc
    blk_id = dst_loc // base.blk
    slot = (dst_loc % base.blk).astype(np.float32)
    win_id = src // base.win
    idx16 = (src % base.win).astype(np.int16)

    nblk, W = base.nblk, base.n_win
    bucket = (core * nblk + blk_id) * W + win_id
    nbuckets = NC * nblk * W
    counts = np.bincount(bucket, minlength=nbuckets)
    cpb = max(1, cdiv(int(counts.max()), P))
    cfg = Cfg(
        n_nodes=base.n_nodes, d=base.d, n_cores=base.n_cores, blk=base.blk,
        win=base.win, cpb=cpb, nb_m=base.nb_m, nb_s=base.nb_s,
    )
    S = cfg.spc

    order = np.argsort(bucket, kind="stable")
    starts = np.zeros(nbuckets + 1, np.int64)
    np.cumsum(counts, out=starts[1:])
    offs = np.arange(E, dtype=np.int64) - starts[bucket[order]]
    pos = bucket[order] * S + offs

    idx_stream = np.full(nbuckets * S, -1, np.int16)
    slot_stream = np.full(nbuckets * S, -1.0, ml_dtypes.bfloat16)
    idx_stream[pos] = idx16[order]
    slot_stream[pos] = slot[order].astype(ml_dtypes.bfloat16)

    cnts = counts.astype(np.int32)
    empty = cnts == 0
    if empty.any():
        # dma_gather needs >= 1 valid idx per call; gather row 0, slot -1.
        idx_stream[np.nonzero(empty)[0] * S] = 0
        cnts[empty] = 1

    # in_norm per core dst range, padded to nblk*blk with zeros and
    # replicated across the 128 partitions (DVE reads need nonzero
    # partition stride).
    dpc_pad = cfg.nblk * cfg.blk
    innorm_all = np.zeros((NC, dpc_pad), np.float32)
    innorm_all[:, :cfg.dpc] = in_norm.reshape(NC, cfg.dpc)

    per_core = cfg.ncalls * S
    in_maps = []
    for c in range(NC):
        seg = slice(c * per_core, (c + 1) * per_core)
        xi = idx_stream[seg].reshape(cfg.ncalls, S // 16, 16)
        xi = np.ascontiguousarray(
            np.tile(xi.transpose(2, 0, 1).reshape(16, -1), (8, 1))
        )
        sl = np.ascontiguousarray(slot_stream[seg].reshape(-1, P).T)
        cn = np.ascontiguousarray(
            cnts[c * cfg.ncalls:(c + 1) * cfg.ncalls].reshape(1, -1)
        )
        inn = np.ascontiguousarray(
            np.broadcast_to(innorm_all[c][None, :], (P, dpc_pad)).copy()
        )
        in_maps.append(
            {"nf": node_f, "idx": xi, "slots": sl, "ncnt": cn, "innorm": inn}
        )
    return cfg, in_maps


def build_nc(cfg: Cfg):
    import concourse.bacc as bacc
    import concourse.mybir as mybir
    from concourse.library_config import mlp

    f32 = mybir.dt.float32
    bf16 = mybir.dt.bfloat16
    D, W, cpb, nblk = cfg.d, cfg.n_win, cfg.cpb, cfg.nblk
    ncalls, nchunks = cfg.ncalls, cfg.nchunks
    idx_cols = ncalls * cfg.spc // 16

    nc = bacc.Bacc("TRN2", target_bir_lowering=False, num_swdge_queues=4)

    nf = nc.dram_tensor("nf", [cfg.n_nodes, D], bf16, kind="ExternalInput")
    idx_d = nc.dram_tensor("idx", [P, idx_cols], mybir.dt.int16, kind="ExternalInput")
    slots_d = nc.dram_tensor("slots", [P, nchunks], bf16, kind="ExternalInput")
    cnt_d = nc.dram_tensor("ncnt", [1, ncalls], mybir.dt.int32, kind="ExternalInput")
    inn_d = nc.dram_tensor("innorm", [P, nblk * cfg.blk], f32, kind="ExternalInput")
    out_d = nc.dram_tensor("out", [P, nblk * cfg.blk], f32, kind="ExternalOutput")

    with ExitStack() as ctx:
        ec = ctx.enter_context
        idx_sb = ec(nc.sbuf_tensor("idx_sb", [P, idx_cols], mybir.dt.int16))
        slots_sb = ec(nc.sbuf_tensor("slots_sb", [P, nchunks], bf16))
        cnt_sb = ec(nc.sbuf_tensor("cnt_sb", [1, ncalls], mybir.dt.int32))
        inn_sb = ec(nc.sbuf_tensor("inn_sb", [P, nblk * cfg.blk], f32))
        iota_sb = ec(nc.sbuf_tensor("iota_sb", [P, cfg.blk], bf16))
        m_sbs = [ec(nc.sbuf_tensor(f"m{j}", [P, cpb, D], bf16)) for j in range(cfg.nb_m)]
        s_sbs = [ec(nc.sbuf_tensor(f"s{j}", [P, cfg.blk], bf16)) for j in range(cfg.nb_s)]
        obufs = [ec(nc.sbuf_tensor(f"ob{j}", [P, cfg.blk], f32)) for j in range(2)]
        psums = [ec(nc.psum_tensor(f"ps{j}", [P, cfg.blk], f32)) for j in range(2)]

        io = ec(nc.semaphore("io"))
        init = ec(nc.semaphore("init"))
        gsems = [ec(nc.semaphore(f"gat{j}")) for j in range(cfg.nb_m)]
        sdve = ec(nc.semaphore("sdve"))
        pe = ec(nc.semaphore("pe"))
        ev = ec(nc.semaphore("ev"))
        osems = [ec(nc.semaphore(f"odma{j}")) for j in range(2)]

        with nc.Block() as block:

            @block.sync
            def _(sync):
                sync.dma_start(idx_sb[:], idx_d[:]).then_inc(io, 16)
                sync.dma_start(slots_sb[:], slots_d[:]).then_inc(io, 16)
                sync.dma_start(cnt_sb[:], cnt_d[:]).then_inc(io, 16)
                sync.dma_start(inn_sb[:], inn_d[:]).then_inc(io, 16)
                for b in range(nblk):
                    sync.wait_ge(ev, b + 1)
                    sync.dma_start(
                        out_d[:, b * cfg.blk:(b + 1) * cfg.blk], obufs[b % 2][:]
                    ).then_inc(osems[b % 2], 16)
                sync.wait_ge(osems[0], 16 * cdiv(nblk, 2))
                if nblk > 1:
                    sync.wait_ge(osems[1], 16 * (nblk // 2))

            @block.gpsimd
            def _(g):
                g.iota(
                    iota_sb[:], [[1, cfg.blk]], channel_multiplier=0,
                    allow_small_or_imprecise_dtypes=True,
                ).then_inc(init, 1)
                for j in range(cfg.nb_m):
                    g.memset(m_sbs[j][:], 0).then_inc(init, 1)
                g.load_library(mlp)
                g.wait_ge(init, 1 + cfg.nb_m)
                g.wait_ge(io, 64)
                with g.register("cnt") as cnt:
                    for k in range(ncalls):
                        w = k % W
                        if k >= cfg.nb_m:
                            g.wait_ge(pe, (k - cfg.nb_m + 1) * cpb)
                        g.reg_load(cnt, cnt_sb[0:1, k:k + 1])
                        rows = min(cfg.win, cfg.n_nodes - w * cfg.win)
                        g.dma_gather(
                            m_sbs[k % cfg.nb_m][:],
                            nf[w * cfg.win: w * cfg.win + rows, :],
                            idx_sb[:, k * cpb * 8:(k + 1) * cpb * 8],
                            cfg.spc,
                            cnt,
                            D,
                            queue_num=k % 4,
                        ).then_inc(gsems[k % cfg.nb_m], 16)

            @block.vector
            def _(v):
                v.wait_ge(io, 64)
                v.wait_ge(init, 1)
                t = 0
                for b in range(nblk):
                    for w in range(W):
                        for i in range(cpb):
                            if t >= cfg.nb_s:
                                v.wait_ge(pe, t - cfg.nb_s + 1)
                            v.tensor_tensor(
                                out=s_sbs[t % cfg.nb_s][:],
                                in0=iota_sb[:],
                                in1=slots_sb[:, t:t + 1].broadcast_to([P, cfg.blk]),
                                op=mybir.AluOpType.is_equal,
                            ).then_inc(sdve, 1)
                            t += 1
                            # psum eviction for block b-1, interleaved right
                            # after the first chunk of block b so S-builds
                            # stay ahead of the PE while the eviction waits.
                            if w == 0 and i == 0 and b >= 1:
                                bb = b - 1
                                v.wait_ge(pe, (bb + 1) * W * cpb)
                                if bb >= 2:
                                    v.wait_ge(osems[bb % 2], 16 * (bb // 2))
                                v.tensor_tensor(
                                    out=obufs[bb % 2][:],
                                    in0=psums[bb % 2][:],
                                    in1=inn_sb[:, bb * cfg.blk:(bb + 1) * cfg.blk],
                                    op=mybir.AluOpType.mult,
                                ).then_inc(ev, 1)
                # final block eviction
                bb = nblk - 1
                v.wait_ge(pe, (bb + 1) * W * cpb)
                if bb >= 2:
                    v.wait_ge(osems[bb % 2], 16 * (bb // 2))
                v.tensor_tensor(
                    out=obufs[bb % 2][:],
                    in0=psums[bb % 2][:],
                    in1=inn_sb[:, bb * cfg.blk:(bb + 1) * cfg.blk],
                    op=mybir.AluOpType.mult,
                ).then_inc(ev, 1)

            @block.tensor
            def _(te):
                t = 0
                for b in range(nblk):
                    for w in range(W):
                        k = b * W + w
                        for i in range(cpb):
                            if i == 0:
                                te.wait_ge(
                                    gsems[k % cfg.nb_m],
                                    16 * (k // cfg.nb_m + 1),
                                )
                            te.wait_ge(sdve, t + 1)
                            start = (w == 0 and i == 0)
                            stop = (w == W - 1 and i == cpb - 1)
                            if start and b >= 2:
                                te.wait_ge(ev, b - 1)
                            te.matmul(
                                psums[b % 2][:],
                                m_sbs[k % cfg.nb_m][:, i, :],
                                s_sbs[t % cfg.nb_s][:],
                                start=start,
                                stop=stop,
                            ).then_inc(pe, 1)
                            t += 1

    nc.compile()
    return nc


def unshard(cfg: Cfg, results):
    out = np.empty((cfg.n_nodes, cfg.d), np.float32)
    for c in range(cfg.n_cores):
        o = results[c]["out"]
        out[c * cfg.dpc:(c + 1) * cfg.dpc, :] = o[:, :cfg.dpc].T
    return out


def run(inputs, trace=False, **spmd_kwargs):
    from concourse.bass_utils import run_bass_kernel_spmd

    cfg, in_maps = prep_host(
        inputs["u_f"], inputs["v_f"], inputs["src"], inputs["dst"]
    )
    nc = build_nc(cfg)
    res = run_bass_kernel_spmd(
        nc, in_maps, core_ids=list(range(cfg.n_cores)), trace=trace,
        **spmd_kwargs,
    )
    return unshard(cfg, res.results), res


def kernel(**inputs):
    return run(inputs)[0]


# revision 3
# speedup vs baseline: 1.4122x; 1.2658x over previous
"""GCN message-passing layer on 8 Trainium2 NeuronCores — v5.

vs v3 (exp3): per-bucket variable chunk counts (max over the 8 cores),
cutting ~14% of the padded matmul/one-hot chunks that static cpb=8 cost.
"""

from contextlib import ExitStack
from dataclasses import dataclass

import numpy as np

P = 128  # SBUF partitions / chunk size (edges per matmul)


def cdiv(a, b):
    return -(-a // b)


@dataclass(frozen=True)
class Cfg:
    n_nodes: int = 100000
    d: int = 128
    n_cores: int = 8
    blk: int = 256      # dst nodes per psum block (matmul N dim)
    win: int = 20000    # src window rows (must be < 32768 for int16 idxs)
    nb_m: int = 6       # gather-destination (M tile) buffers
    nb_s: int = 8       # one-hot (S tile) buffers
    cpbk: tuple = ()    # chunks per bucket (shared across cores), from data

    @property
    def dpc(self):  # dst nodes per core
        return self.n_nodes // self.n_cores

    @property
    def nblk(self):  # blocks per core
        return cdiv(self.dpc, self.blk)

    @property
    def n_win(self):
        return cdiv(self.n_nodes, self.win)

    @property
    def ncalls(self):  # gather calls per core (= buckets per core)
        return self.nblk * self.n_win

    @property
    def nchunks(self):
        return sum(self.cpbk)

    @property
    def cpb_max(self):
        return max(self.cpbk)


def prep_host(u_f, v_f, src, dst, cfg: Cfg | None = None):
    """Bucket/pad edges; returns (cfg, per-core input maps)."""
    import ml_dtypes

    u_f = np.asarray(u_f, dtype=np.float32)
    v_f = np.asarray(v_f, dtype=np.float32)
    src = np.asarray(src).astype(np.int64)
    dst = np.asarray(dst).astype(np.int64)
    base = cfg or Cfg()
    N, NC = base.n_nodes, base.n_cores
    E = src.shape[0]

    deg_out = np.bincount(src, minlength=N).astype(np.float32)
    deg_in = np.bincount(dst, minlength=N).astype(np.float32)
    out_norm = np.power(np.clip(deg_out, 1.0, None), np.float32(-0.5))
    in_norm = np.power(np.clip(deg_in, 1.0, None), np.float32(-0.5))

    node_f = np.concatenate([u_f, v_f], axis=0) * out_norm[:, None]
    node_f = np.ascontiguousarray(node_f.astype(ml_dtypes.bfloat16))
    assert node_f.shape == (N, base.d)

    core = dst // base.dpc
    dst_loc = dst % base.dpc
    blk_id = dst_loc // base.blk
    slot = (dst_loc % base.blk).astype(np.float32)
    win_id = src // base.win
    idx16 = (src % base.win).astype(np.int16)

    nblk, W = base.nblk, base.n_win
    ncalls = nblk * W
    bucket_local = blk_id * W + win_id           # bucket id within a core
    bucket = core * ncalls + bucket_local
    nbuckets = NC * ncalls
    counts = np.bincount(bucket, minlength=nbuckets).reshape(NC, ncalls)
    # The SWDGE descriptor ring holds ~1024 descriptors; one gather call
    # per bucket requires every bucket to stay under that.
    assert counts.max() <= 1024, (
        f"bucket overflow: {counts.max()} edges > 1024; reduce cfg.win"
    )
    # chunks per bucket: shared across cores (SPMD single program)
    cpbk = np.maximum(1, cdiv(counts.max(axis=0), P))  # [ncalls]
    cfg = Cfg(
        n_nodes=base.n_nodes, d=base.d, n_cores=base.n_cores, blk=base.blk,
        win=base.win, nb_m=base.nb_m, nb_s=base.nb_s,
        cpbk=tuple(int(x) for x in cpbk),
    )
    spck = cpbk * P                               # padded slots per bucket
    bucket_off = np.zeros(ncalls + 1, np.int64)   # slot offset per bucket
    np.cumsum(spck, out=bucket_off[1:])
    total_slots = int(bucket_off[-1])

    order = np.argsort(bucket, kind="stable")
    starts = np.zeros(nbuckets + 1, np.int64)
    np.cumsum(counts.reshape(-1), out=starts[1:])
    offs = np.arange(E, dtype=np.int64) - starts[bucket[order]]
    bloc = bucket_local[order]
    cor = bucket[order] // ncalls
    pos = cor * total_slots + bucket_off[bloc] + offs

    idx_stream = np.full(NC * total_slots, -1, np.int16)
    slot_stream = np.full(NC * total_slots, -1.0, ml_dtypes.bfloat16)
    idx_stream[pos] = idx16[order]
    slot_stream[pos] = slot[order].astype(ml_dtypes.bfloat16)

    cnts = counts.astype(np.int32)  # [NC, ncalls]
    empty = cnts == 0
    if empty.any():
        # dma_gather needs >= 1 valid idx per call; gather row 0, slot -1.
        ec, ek = np.nonzero(empty)
        idx_stream[ec * total_slots + bucket_off[ek]] = 0
        cnts[empty] = 1

    # in_norm per core dst range, padded to nblk*blk with zeros and
    # replicated across the 128 partitions (DVE needs nonzero partition
    # stride).
    dpc_pad = cfg.nblk * cfg.blk
    innorm_all = np.zeros((NC, dpc_pad), np.float32)
    innorm_all[:, :cfg.dpc] = in_norm.reshape(NC, cfg.dpc)

    in_maps = []
    for c in range(NC):
        seg = slice(c * total_slots, (c + 1) * total_slots)
        xi = idx_stream[seg].reshape(total_slots // 16, 16)
        xi = np.ascontiguousarray(np.tile(xi.T, (8, 1)))
        sl = np.ascontiguousarray(slot_stream[seg].reshape(-1, P).T)
        cn = np.ascontiguousarray(cnts[c].reshape(1, -1))
        inn = np.ascontiguousarray(
            np.broadcast_to(innorm_all[c][None, :], (P, dpc_pad)).copy()
        )
        in_maps.append(
            {"nf": node_f, "idx": xi, "slots": sl, "ncnt": cn, "innorm": inn}
        )
    return cfg, in_maps


def build_nc(cfg: Cfg):
    import concourse.bacc as bacc
    import concourse.mybir as mybir
    from concourse.library_config import mlp

    f32 = mybir.dt.float32
    bf16 = mybir.dt.bfloat16
    D, W, nblk = cfg.d, cfg.n_win, cfg.nblk
    ncalls, nchunks = cfg.ncalls, cfg.nchunks
    cpbk = cfg.cpbk
    cum = [0]
    for x in cpbk:
        cum.append(cum[-1] + x)  # cumulative chunks before bucket k
    idx_cols = nchunks * P // 16

    nc = bacc.Bacc("TRN2", target_bir_lowering=False, num_swdge_queues=4)

    nf = nc.dram_tensor("nf", [cfg.n_nodes, D], bf16, kind="ExternalInput")
    idx_d = nc.dram_tensor("idx", [P, idx_cols], mybir.dt.int16, kind="ExternalInput")
    slots_d = nc.dram_tensor("slots", [P, nchunks], bf16, kind="ExternalInput")
    cnt_d = nc.dram_tensor("ncnt", [1, ncalls], mybir.dt.int32, kind="ExternalInput")
    inn_d = nc.dram_tensor("innorm", [P, nblk * cfg.blk], f32, kind="ExternalInput")
    out_d = nc.dram_tensor("out", [P, nblk * cfg.blk], f32, kind="ExternalOutput")

    with ExitStack() as ctx:
        ec = ctx.enter_context
        idx_sb = ec(nc.sbuf_tensor("idx_sb", [P, idx_cols], mybir.dt.int16))
        slots_sb = ec(nc.sbuf_tensor("slots_sb", [P, nchunks], bf16))
        cnt_sb = ec(nc.sbuf_tensor("cnt_sb", [1, ncalls], mybir.dt.int32))
        inn_sb = ec(nc.sbuf_tensor("inn_sb", [P, nblk * cfg.blk], f32))
        iota_sb = ec(nc.sbuf_tensor("iota_sb", [P, cfg.blk], bf16))
        m_sbs = [
            ec(nc.sbuf_tensor(f"m{j}", [P, cfg.cpb_max, D], bf16))
            for j in range(cfg.nb_m)
        ]
        s_sbs = [ec(nc.sbuf_tensor(f"s{j}", [P, cfg.blk], bf16)) for j in range(cfg.nb_s)]
        obufs = [ec(nc.sbuf_tensor(f"ob{j}", [P, cfg.blk], f32)) for j in range(2)]
        psums = [ec(nc.psum_tensor(f"ps{j}", [P, cfg.blk], f32)) for j in range(2)]

        io = ec(nc.semaphore("io"))
        init = ec(nc.semaphore("init"))
        gsems = [ec(nc.semaphore(f"gat{j}")) for j in range(cfg.nb_m)]
        sdve = ec(nc.semaphore("sdve"))
        pe = ec(nc.semaphore("pe"))
        ev = ec(nc.semaphore("ev"))
        osems = [ec(nc.semaphore(f"odma{j}")) for j in range(2)]

        def evict(v, bb):
            v.wait_ge(pe, cum[(bb + 1) * W])
            if bb >= 2:
                v.wait_ge(osems[bb % 2], 16 * (bb // 2))
            v.tensor_tensor(
                out=obufs[bb % 2][:],
                in0=psums[bb % 2][:],
                in1=inn_sb[:, bb * cfg.blk:(bb + 1) * cfg.blk],
                op=mybir.AluOpType.mult,
            ).then_inc(ev, 1)

        with nc.Block() as block:

            @block.sync
            def _(sync):
                sync.dma_start(idx_sb[:], idx_d[:]).then_inc(io, 16)
                sync.dma_start(slots_sb[:], slots_d[:]).then_inc(io, 16)
                sync.dma_start(cnt_sb[:], cnt_d[:]).then_inc(io, 16)
                sync.dma_start(inn_sb[:], inn_d[:]).then_inc(io, 16)
                for b in range(nblk):
                    sync.wait_ge(ev, b + 1)
                    sync.dma_start(
                        out_d[:, b * cfg.blk:(b + 1) * cfg.blk], obufs[b % 2][:]
                    ).then_inc(osems[b % 2], 16)
                sync.wait_ge(osems[0], 16 * cdiv(nblk, 2))
                if nblk > 1:
                    sync.wait_ge(osems[1], 16 * (nblk // 2))

            @block.gpsimd
            def _(g):
                g.iota(
                    iota_sb[:], [[1, cfg.blk]], channel_multiplier=0,
                    allow_small_or_imprecise_dtypes=True,
                ).then_inc(init, 1)
                for j in range(cfg.nb_m):
                    g.memset(m_sbs[j][:], 0).then_inc(init, 1)
                g.load_library(mlp)
                g.wait_ge(init, 1 + cfg.nb_m)
                g.wait_ge(io, 64)
                with g.register("cnt") as cnt:
                    for k in range(ncalls):
                        w = k % W
                        if k >= cfg.nb_m:
                            # buffer k%nb_m reused: bucket k-nb_m consumed
                            g.wait_ge(pe, cum[k - cfg.nb_m + 1])
                        g.reg_load(cnt, cnt_sb[0:1, k:k + 1])
                        rows = min(cfg.win, cfg.n_nodes - w * cfg.win)
                        g.dma_gather(
                            m_sbs[k % cfg.nb_m][:, :cpbk[k], :],
                            nf[w * cfg.win: w * cfg.win + rows, :],
                            idx_sb[:, cum[k] * 8:cum[k + 1] * 8],
                            cpbk[k] * P,
                            cnt,
                            D,
                            queue_num=k % 4,
                        ).then_inc(gsems[k % cfg.nb_m], 16)

            @block.vector
            def _(v):
                v.wait_ge(io, 64)
                v.wait_ge(init, 1)
                t = 0
                for b in range(nblk):
                    for w in range(W):
                        k = b * W + w
                        for i in range(cpbk[k]):
                            if t >= cfg.nb_s:
                                v.wait_ge(pe, t - cfg.nb_s + 1)
                            v.tensor_tensor(
                                out=s_sbs[t % cfg.nb_s][:],
                                in0=iota_sb[:],
                                in1=slots_sb[:, t:t + 1].broadcast_to([P, cfg.blk]),
                                op=mybir.AluOpType.is_equal,
                            ).then_inc(sdve, 1)
                            t += 1
                            # psum eviction for block b-1, interleaved after
                            # the first chunk of block b so S-builds stay
                            # ahead of the PE while the eviction waits.
                            if w == 0 and i == 0 and b >= 1:
                                evict(v, b - 1)
                evict(v, nblk - 1)

            @block.tensor
            def _(te):
                t = 0
                for b in range(nblk):
                    for w in range(W):
                        k = b * W + w
                        for i in range(cpbk[k]):
                            if i == 0:
                                te.wait_ge(
                                    gsems[k % cfg.nb_m],
                                    16 * (k // cfg.nb_m + 1),
                                )
                            te.wait_ge(sdve, t + 1)
                            start = (w == 0 and i == 0)
                            stop = (w == W - 1 and i == cpbk[k] - 1)
                            if start and b >= 2:
                                te.wait_ge(ev, b - 1)
                            te.matmul(
                                psums[b % 2][:],
                                m_sbs[k % cfg.nb_m][:, i, :],
                                s_sbs[t % cfg.nb_s][:],
                                start=start,
                                stop=stop,
                            ).then_inc(pe, 1)
                            t += 1

    nc.compile()
    return nc


def unshard(cfg: Cfg, results):
    out = np.empty((cfg.n_nodes, cfg.d), np.float32)
    for c in range(cfg.n_cores):
        o = results[c]["out"]
        out[c * cfg.dpc:(c + 1) * cfg.dpc, :] = o[:, :cfg.dpc].T
    return out


def run(inputs, trace=False, **spmd_kwargs):
    from concourse.bass_utils import run_bass_kernel_spmd

    cfg, in_maps = prep_host(
        inputs["u_f"], inputs["v_f"], inputs["src"], inputs["dst"]
    )
    nc = build_nc(cfg)
    res = run_bass_kernel_spmd(
        nc, in_maps, core_ids=list(range(cfg.n_cores)), trace=trace,
        **spmd_kwargs,
    )
    return unshard(cfg, res.results), res


def kernel(**inputs):
    return run(inputs)[0]


# revision 4
# speedup vs baseline: 1.4191x; 1.0049x over previous
"""GCN message-passing layer on 8 Trainium2 NeuronCores — v7.

vs v5: every 3rd one-hot S build moves to the (otherwise idle) scalar
engine as Relu(1-|slot-iota|) in two activation passes, cutting the DVE
cadence that governs the whole pipeline.
"""

from contextlib import ExitStack
from dataclasses import dataclass

import numpy as np

P = 128  # SBUF partitions / chunk size (edges per matmul)


def cdiv(a, b):
    return -(-a // b)


@dataclass(frozen=True)
class Cfg:
    n_nodes: int = 100000
    d: int = 128
    n_cores: int = 8
    blk: int = 256      # dst nodes per psum block (matmul N dim)
    win: int = 20000    # src window rows (must be < 32768 for int16 idxs)
    nb_m: int = 6       # gather-destination (M tile) buffers
    nb_s: int = 8       # one-hot (S tile) buffers
    cpbk: tuple = ()    # chunks per bucket (shared across cores), from data

    @property
    def dpc(self):  # dst nodes per core
        return self.n_nodes // self.n_cores

    @property
    def nblk(self):  # blocks per core
        return cdiv(self.dpc, self.blk)

    @property
    def n_win(self):
        return cdiv(self.n_nodes, self.win)

    @property
    def ncalls(self):  # gather calls per core (= buckets per core)
        return self.nblk * self.n_win

    @property
    def nchunks(self):
        return sum(self.cpbk)

    @property
    def cpb_max(self):
        return max(self.cpbk)


def prep_host(u_f, v_f, src, dst, cfg: Cfg | None = None):
    """Bucket/pad edges; returns (cfg, per-core input maps)."""
    import ml_dtypes

    u_f = np.asarray(u_f, dtype=np.float32)
    v_f = np.asarray(v_f, dtype=np.float32)
    src = np.asarray(src).astype(np.int64)
    dst = np.asarray(dst).astype(np.int64)
    base = cfg or Cfg()
    N, NC = base.n_nodes, base.n_cores
    E = src.shape[0]

    deg_out = np.bincount(src, minlength=N).astype(np.float32)
    deg_in = np.bincount(dst, minlength=N).astype(np.float32)
    out_norm = np.power(np.clip(deg_out, 1.0, None), np.float32(-0.5))
    in_norm = np.power(np.clip(deg_in, 1.0, None), np.float32(-0.5))

    node_f = np.concatenate([u_f, v_f], axis=0) * out_norm[:, None]
    node_f = np.ascontiguousarray(node_f.astype(ml_dtypes.bfloat16))
    assert node_f.shape == (N, base.d)

    core = dst // base.dpc
    dst_loc = dst % base.dpc
    blk_id = dst_loc // base.blk
    slot = (dst_loc % base.blk).astype(np.float32)
    win_id = src // base.win
    idx16 = (src % base.win).astype(np.int16)

    nblk, W = base.nblk, base.n_win
    ncalls = nblk * W
    bucket_local = blk_id * W + win_id           # bucket id within a core
    bucket = core * ncalls + bucket_local
    nbuckets = NC * ncalls
    counts = np.bincount(bucket, minlength=nbuckets).reshape(NC, ncalls)
    # The SWDGE descriptor ring holds ~1024 descriptors; one gather call
    # per bucket requires every bucket to stay under that.
    assert counts.max() <= 1024, (
        f"bucket overflow: {counts.max()} edges > 1024; reduce cfg.win"
    )
    # chunks per bucket: shared across cores (SPMD single program)
    cpbk = np.maximum(1, cdiv(counts.max(axis=0), P))  # [ncalls]
    cfg = Cfg(
        n_nodes=base.n_nodes, d=base.d, n_cores=base.n_cores, blk=base.blk,
        win=base.win, nb_m=base.nb_m, nb_s=base.nb_s,
        cpbk=tuple(int(x) for x in cpbk),
    )
    spck = cpbk * P                               # padded slots per bucket
    bucket_off = np.zeros(ncalls + 1, np.int64)   # slot offset per bucket
    np.cumsum(spck, out=bucket_off[1:])
    total_slots = int(bucket_off[-1])

    order = np.argsort(bucket, kind="stable")
    starts = np.zeros(nbuckets + 1, np.int64)
    np.cumsum(counts.reshape(-1), out=starts[1:])
    offs = np.arange(E, dtype=np.int64) - starts[bucket[order]]
    bloc = bucket_local[order]
    cor = bucket[order] // ncalls
    pos = cor * total_slots + bucket_off[bloc] + offs

    idx_stream = np.full(NC * total_slots, -1, np.int16)
    slot_stream = np.full(NC * total_slots, -1.0, ml_dtypes.bfloat16)
    slot_stream32 = np.full(NC * total_slots, -1.0, np.float32)
    idx_stream[pos] = idx16[order]
    slot_stream[pos] = slot[order].astype(ml_dtypes.bfloat16)
    slot_stream32[pos] = slot[order]

    cnts = counts.astype(np.int32)  # [NC, ncalls]
    empty = cnts == 0
    if empty.any():
        # dma_gather needs >= 1 valid idx per call; gather row 0, slot -1.
        ec, ek = np.nonzero(empty)
        idx_stream[ec * total_slots + bucket_off[ek]] = 0
        cnts[empty] = 1

    # in_norm per core dst range, padded to nblk*blk with zeros and
    # replicated across the 128 partitions (DVE needs nonzero partition
    # stride).
    dpc_pad = cfg.nblk * cfg.blk
    innorm_all = np.zeros((NC, dpc_pad), np.float32)
    innorm_all[:, :cfg.dpc] = in_norm.reshape(NC, cfg.dpc)

    in_maps = []
    for c in range(NC):
        seg = slice(c * total_slots, (c + 1) * total_slots)
        xi = idx_stream[seg].reshape(total_slots // 16, 16)
        xi = np.ascontiguousarray(np.tile(xi.T, (8, 1)))
        sl = np.ascontiguousarray(slot_stream[seg].reshape(-1, P).T)
        sl32 = np.ascontiguousarray(slot_stream32[seg].reshape(-1, P).T)
        cn = np.ascontiguousarray(cnts[c].reshape(1, -1))
        inn = np.ascontiguousarray(
            np.broadcast_to(innorm_all[c][None, :], (P, dpc_pad)).copy()
        )
        in_maps.append(
            {"nf": node_f, "idx": xi, "slots": sl, "slots32": sl32,
             "ncnt": cn, "innorm": inn}
        )
    return cfg, in_maps


def build_nc(cfg: Cfg):
    import concourse.bacc as bacc
    import concourse.mybir as mybir
    from concourse.library_config import mlp

    f32 = mybir.dt.float32
    bf16 = mybir.dt.bfloat16
    D, W, nblk = cfg.d, cfg.n_win, cfg.nblk
    ncalls, nchunks = cfg.ncalls, cfg.nchunks
    cpbk = cfg.cpbk
    cum = [0]
    for x in cpbk:
        cum.append(cum[-1] + x)  # cumulative chunks before bucket k
    idx_cols = nchunks * P // 16

    nc = bacc.Bacc("TRN2", target_bir_lowering=False, num_swdge_queues=4)

    nf = nc.dram_tensor("nf", [cfg.n_nodes, D], bf16, kind="ExternalInput")
    idx_d = nc.dram_tensor("idx", [P, idx_cols], mybir.dt.int16, kind="ExternalInput")
    slots_d = nc.dram_tensor("slots", [P, nchunks], bf16, kind="ExternalInput")
    slots32_d = nc.dram_tensor("slots32", [P, nchunks], f32, kind="ExternalInput")
    cnt_d = nc.dram_tensor("ncnt", [1, ncalls], mybir.dt.int32, kind="ExternalInput")
    inn_d = nc.dram_tensor("innorm", [P, nblk * cfg.blk], f32, kind="ExternalInput")
    out_d = nc.dram_tensor("out", [P, nblk * cfg.blk], f32, kind="ExternalOutput")

    with ExitStack() as ctx:
        ec = ctx.enter_context
        idx_sb = ec(nc.sbuf_tensor("idx_sb", [P, idx_cols], mybir.dt.int16))
        slots_sb = ec(nc.sbuf_tensor("slots_sb", [P, nchunks], bf16))
        slots32_sb = ec(nc.sbuf_tensor("slots32_sb", [P, nchunks], f32))
        tmp_act = ec(nc.sbuf_tensor("tmp_act", [P, cfg.blk], bf16))
        cnt_sb = ec(nc.sbuf_tensor("cnt_sb", [1, ncalls], mybir.dt.int32))
        inn_sb = ec(nc.sbuf_tensor("inn_sb", [P, nblk * cfg.blk], f32))
        iota_sb = ec(nc.sbuf_tensor("iota_sb", [P, cfg.blk], bf16))
        m_sbs = [
            ec(nc.sbuf_tensor(f"m{j}", [P, cfg.cpb_max, D], bf16))
            for j in range(cfg.nb_m)
        ]
        s_sbs = [ec(nc.sbuf_tensor(f"s{j}", [P, cfg.blk], bf16)) for j in range(cfg.nb_s)]
        obufs = [ec(nc.sbuf_tensor(f"ob{j}", [P, cfg.blk], f32)) for j in range(2)]
        psums = [ec(nc.psum_tensor(f"ps{j}", [P, cfg.blk], f32)) for j in range(2)]

        io = ec(nc.semaphore("io"))
        init = ec(nc.semaphore("init"))
        gsems = [ec(nc.semaphore(f"gat{j}")) for j in range(cfg.nb_m)]
        sdve = ec(nc.semaphore("sdve"))
        sact = ec(nc.semaphore("sact"))
        pe = ec(nc.semaphore("pe"))
        ev = ec(nc.semaphore("ev"))
        osems = [ec(nc.semaphore(f"odma{j}")) for j in range(2)]

        ACT_EVERY = 3  # chunk t built on ACT iff t % ACT_EVERY == 2

        def evict(v, bb):
            v.wait_ge(pe, cum[(bb + 1) * W])
            if bb >= 2:
                v.wait_ge(osems[bb % 2], 16 * (bb // 2))
            v.tensor_tensor(
                out=obufs[bb % 2][:],
                in0=psums[bb % 2][:],
                in1=inn_sb[:, bb * cfg.blk:(bb + 1) * cfg.blk],
                op=mybir.AluOpType.mult,
            ).then_inc(ev, 1)

        with nc.Block() as block:

            @block.sync
            def _(sync):
                sync.dma_start(idx_sb[:], idx_d[:]).then_inc(io, 16)
                sync.dma_start(slots_sb[:], slots_d[:]).then_inc(io, 16)
                sync.dma_start(slots32_sb[:], slots32_d[:]).then_inc(io, 16)
                sync.dma_start(cnt_sb[:], cnt_d[:]).then_inc(io, 16)
                sync.dma_start(inn_sb[:], inn_d[:]).then_inc(io, 16)
                for b in range(nblk):
                    sync.wait_ge(ev, b + 1)
                    sync.dma_start(
                        out_d[:, b * cfg.blk:(b + 1) * cfg.blk], obufs[b % 2][:]
                    ).then_inc(osems[b % 2], 16)
                sync.wait_ge(osems[0], 16 * cdiv(nblk, 2))
                if nblk > 1:
                    sync.wait_ge(osems[1], 16 * (nblk // 2))

            @block.gpsimd
            def _(g):
                g.iota(
                    iota_sb[:], [[1, cfg.blk]], channel_multiplier=0,
                    allow_small_or_imprecise_dtypes=True,
                ).then_inc(init, 1)
                for j in range(cfg.nb_m):
                    g.memset(m_sbs[j][:], 0).then_inc(init, 1)
                g.load_library(mlp)
                g.wait_ge(init, 1 + cfg.nb_m)
                g.wait_ge(io, 80)
                with g.register("cnt") as cnt:
                    for k in range(ncalls):
                        w = k % W
                        if k >= cfg.nb_m:
                            # buffer k%nb_m reused: bucket k-nb_m consumed
                            g.wait_ge(pe, cum[k - cfg.nb_m + 1])
                        g.reg_load(cnt, cnt_sb[0:1, k:k + 1])
                        rows = min(cfg.win, cfg.n_nodes - w * cfg.win)
                        g.dma_gather(
                            m_sbs[k % cfg.nb_m][:, :cpbk[k], :],
                            nf[w * cfg.win: w * cfg.win + rows, :],
                            idx_sb[:, cum[k] * 8:cum[k + 1] * 8],
                            cpbk[k] * P,
                            cnt,
                            D,
                            queue_num=k % 4,
                        ).then_inc(gsems[k % cfg.nb_m], 16)

            @block.vector
            def _(v):
                v.wait_ge(io, 80)
                v.wait_ge(init, 1)
                t = 0
                nd = 0
                for b in range(nblk):
                    for w in range(W):
                        k = b * W + w
                        for i in range(cpbk[k]):
                            if t % ACT_EVERY != 2:
                                if t >= cfg.nb_s:
                                    v.wait_ge(pe, t - cfg.nb_s + 1)
                                v.tensor_tensor(
                                    out=s_sbs[t % cfg.nb_s][:],
                                    in0=iota_sb[:],
                                    in1=slots_sb[:, t:t + 1].broadcast_to([P, cfg.blk]),
                                    op=mybir.AluOpType.is_equal,
                                ).then_inc(sdve, 1)
                                nd += 1
                            t += 1
                            # psum eviction for block b-1, interleaved after
                            # the first chunk of block b so S-builds stay
                            # ahead of the PE while the eviction waits.
                            if w == 0 and i == 0 and b >= 1:
                                evict(v, b - 1)
                evict(v, nblk - 1)

            @block.scalar
            def _(a):
                a.wait_ge(io, 80)
                a.wait_ge(init, 1)
                t = 0
                for b in range(nblk):
                    for w in range(W):
                        k = b * W + w
                        for i in range(cpbk[k]):
                            if t % ACT_EVERY == 2:
                                if t >= cfg.nb_s:
                                    a.wait_ge(pe, t - cfg.nb_s + 1)
                                a.activation(
                                    tmp_act[:], iota_sb[:],
                                    mybir.ActivationFunctionType.Abs,
                                    bias=slots32_sb[:, t:t + 1],
                                    scale=-1.0,
                                )
                                a.activation(
                                    s_sbs[t % cfg.nb_s][:], tmp_act[:],
                                    mybir.ActivationFunctionType.Relu,
                                    bias=1.0,
                                    scale=-1.0,
                                ).then_inc(sact, 1)
                            t += 1

            @block.tensor
            def _(te):
                t = 0
                ndve = 0
                nact = 0
                for b in range(nblk):
                    for w in range(W):
                        k = b * W + w
                        for i in range(cpbk[k]):
                            if i == 0:
                                te.wait_ge(
                                    gsems[k % cfg.nb_m],
                                    16 * (k // cfg.nb_m + 1),
                                )
                            if t % ACT_EVERY == 2:
                                nact += 1
                                te.wait_ge(sact, nact)
                            else:
                                ndve += 1
                                te.wait_ge(sdve, ndve)
                            start = (w == 0 and i == 0)
                            stop = (w == W - 1 and i == cpbk[k] - 1)
                            if start and b >= 2:
                                te.wait_ge(ev, b - 1)
                            te.matmul(
                                psums[b % 2][:],
                                m_sbs[k % cfg.nb_m][:, i, :],
                                s_sbs[t % cfg.nb_s][:],
                                start=start,
                                stop=stop,
                            ).then_inc(pe, 1)
                            t += 1

    nc.compile()
    return nc


def unshard(cfg: Cfg, results):
    out = np.empty((cfg.n_nodes, cfg.d), np.float32)
    for c in range(cfg.n_cores):
        o = results[c]["out"]
        out[c * cfg.dpc:(c + 1) * cfg.dpc, :] = o[:, :cfg.dpc].T
    return out


def run(inputs, trace=False, **spmd_kwargs):
    from concourse.bass_utils import run_bass_kernel_spmd

    cfg, in_maps = prep_host(
        inputs["u_f"], inputs["v_f"], inputs["src"], inputs["dst"]
    )
    nc = build_nc(cfg)
    res = run_bass_kernel_spmd(
        nc, in_maps, core_ids=list(range(cfg.n_cores)), trace=trace,
        **spmd_kwargs,
    )
    return unshard(cfg, res.results), res


def kernel(**inputs):
    return run(inputs)[0]


# revision 5
# speedup vs baseline: 1.4252x; 1.0043x over previous
"""GCN message-passing layer on 8 Trainium2 NeuronCores — v9.

vs v7: DVE builds one-hots in groups of 4 per tensor_tensor (3-D APs:
repeated iota vs stride-0-broadcast slot ids), amortizing per-op
overhead; the scalar engine takes every 4th group (as four 2-pass
Relu(1-|slot-iota|) builds) for ~25% offload.
"""

from contextlib import ExitStack
from dataclasses import dataclass

import numpy as np

P = 128  # SBUF partitions / chunk size (edges per matmul)


def cdiv(a, b):
    return -(-a // b)


@dataclass(frozen=True)
class Cfg:
    n_nodes: int = 100000
    d: int = 128
    n_cores: int = 8
    blk: int = 256      # dst nodes per psum block (matmul N dim)
    win: int = 20000    # src window rows (must be < 32768 for int16 idxs)
    nb_m: int = 6       # gather-destination (M tile) buffers
    nb_s: int = 4       # one-hot group buffers (4 chunks each)
    cpbk: tuple = ()    # chunks per bucket (shared across cores), from data

    @property
    def dpc(self):  # dst nodes per core
        return self.n_nodes // self.n_cores

    @property
    def nblk(self):  # blocks per core
        return cdiv(self.dpc, self.blk)

    @property
    def n_win(self):
        return cdiv(self.n_nodes, self.win)

    @property
    def ncalls(self):  # gather calls per core (= buckets per core)
        return self.nblk * self.n_win

    @property
    def nchunks(self):
        return sum(self.cpbk)

    @property
    def cpb_max(self):
        return max(self.cpbk)


def prep_host(u_f, v_f, src, dst, cfg: Cfg | None = None):
    """Bucket/pad edges; returns (cfg, per-core input maps)."""
    import ml_dtypes

    u_f = np.asarray(u_f, dtype=np.float32)
    v_f = np.asarray(v_f, dtype=np.float32)
    src = np.asarray(src).astype(np.int64)
    dst = np.asarray(dst).astype(np.int64)
    base = cfg or Cfg()
    N, NC = base.n_nodes, base.n_cores
    E = src.shape[0]

    deg_out = np.bincount(src, minlength=N).astype(np.float32)
    deg_in = np.bincount(dst, minlength=N).astype(np.float32)
    out_norm = np.power(np.clip(deg_out, 1.0, None), np.float32(-0.5))
    in_norm = np.power(np.clip(deg_in, 1.0, None), np.float32(-0.5))

    node_f = np.concatenate([u_f, v_f], axis=0) * out_norm[:, None]
    node_f = np.ascontiguousarray(node_f.astype(ml_dtypes.bfloat16))
    assert node_f.shape == (N, base.d)

    core = dst // base.dpc
    dst_loc = dst % base.dpc
    blk_id = dst_loc // base.blk
    slot = (dst_loc % base.blk).astype(np.float32)
    win_id = src // base.win
    idx16 = (src % base.win).astype(np.int16)

    nblk, W = base.nblk, base.n_win
    ncalls = nblk * W
    bucket_local = blk_id * W + win_id           # bucket id within a core
    bucket = core * ncalls + bucket_local
    nbuckets = NC * ncalls
    counts = np.bincount(bucket, minlength=nbuckets).reshape(NC, ncalls)
    # The SWDGE descriptor ring holds ~1024 descriptors; one gather call
    # per bucket requires every bucket to stay under that.
    assert counts.max() <= 1024, (
        f"bucket overflow: {counts.max()} edges > 1024; reduce cfg.win"
    )
    # chunks per bucket: shared across cores (SPMD single program)
    cpbk = np.maximum(1, cdiv(counts.max(axis=0), P))  # [ncalls]
    cfg = Cfg(
        n_nodes=base.n_nodes, d=base.d, n_cores=base.n_cores, blk=base.blk,
        win=base.win, nb_m=base.nb_m, nb_s=base.nb_s,
        cpbk=tuple(int(x) for x in cpbk),
    )
    spck = cpbk * P                               # padded slots per bucket
    bucket_off = np.zeros(ncalls + 1, np.int64)   # slot offset per bucket
    np.cumsum(spck, out=bucket_off[1:])
    total_slots = int(bucket_off[-1])

    order = np.argsort(bucket, kind="stable")
    starts = np.zeros(nbuckets + 1, np.int64)
    np.cumsum(counts.reshape(-1), out=starts[1:])
    offs = np.arange(E, dtype=np.int64) - starts[bucket[order]]
    bloc = bucket_local[order]
    cor = bucket[order] // ncalls
    pos = cor * total_slots + bucket_off[bloc] + offs

    idx_stream = np.full(NC * total_slots, -1, np.int16)
    slot_stream = np.full(NC * total_slots, -1.0, ml_dtypes.bfloat16)
    slot_stream32 = np.full(NC * total_slots, -1.0, np.float32)
    idx_stream[pos] = idx16[order]
    slot_stream[pos] = slot[order].astype(ml_dtypes.bfloat16)
    slot_stream32[pos] = slot[order]

    cnts = counts.astype(np.int32)  # [NC, ncalls]
    empty = cnts == 0
    if empty.any():
        # dma_gather needs >= 1 valid idx per call; gather row 0, slot -1.
        ec, ek = np.nonzero(empty)
        idx_stream[ec * total_slots + bucket_off[ek]] = 0
        cnts[empty] = 1

    # in_norm per core dst range, padded to nblk*blk with zeros and
    # replicated across the 128 partitions (DVE needs nonzero partition
    # stride).
    dpc_pad = cfg.nblk * cfg.blk
    innorm_all = np.zeros((NC, dpc_pad), np.float32)
    innorm_all[:, :cfg.dpc] = in_norm.reshape(NC, cfg.dpc)

    in_maps = []
    for c in range(NC):
        seg = slice(c * total_slots, (c + 1) * total_slots)
        xi = idx_stream[seg].reshape(total_slots // 16, 16)
        xi = np.ascontiguousarray(np.tile(xi.T, (8, 1)))
        sl = np.ascontiguousarray(slot_stream[seg].reshape(-1, P).T)
        sl32 = np.ascontiguousarray(slot_stream32[seg].reshape(-1, P).T)
        cn = np.ascontiguousarray(cnts[c].reshape(1, -1))
        inn = np.ascontiguousarray(
            np.broadcast_to(innorm_all[c][None, :], (P, dpc_pad)).copy()
        )
        in_maps.append(
            {"nf": node_f, "idx": xi, "slots": sl, "slots32": sl32,
             "ncnt": cn, "innorm": inn}
        )
    return cfg, in_maps


def build_nc(cfg: Cfg):
    import concourse.bacc as bacc
    import concourse.mybir as mybir
    from concourse.library_config import mlp

    f32 = mybir.dt.float32
    bf16 = mybir.dt.bfloat16
    D, W, nblk = cfg.d, cfg.n_win, cfg.nblk
    ncalls, nchunks = cfg.ncalls, cfg.nchunks
    cpbk = cfg.cpbk
    cum = [0]
    for x in cpbk:
        cum.append(cum[-1] + x)  # cumulative chunks before bucket k
    idx_cols = nchunks * P // 16

    nc = bacc.Bacc("TRN2", target_bir_lowering=False, num_swdge_queues=4)

    nf = nc.dram_tensor("nf", [cfg.n_nodes, D], bf16, kind="ExternalInput")
    idx_d = nc.dram_tensor("idx", [P, idx_cols], mybir.dt.int16, kind="ExternalInput")
    slots_d = nc.dram_tensor("slots", [P, nchunks], bf16, kind="ExternalInput")
    slots32_d = nc.dram_tensor("slots32", [P, nchunks], f32, kind="ExternalInput")
    cnt_d = nc.dram_tensor("ncnt", [1, ncalls], mybir.dt.int32, kind="ExternalInput")
    inn_d = nc.dram_tensor("innorm", [P, nblk * cfg.blk], f32, kind="ExternalInput")
    out_d = nc.dram_tensor("out", [P, nblk * cfg.blk], f32, kind="ExternalOutput")

    with ExitStack() as ctx:
        ec = ctx.enter_context
        idx_sb = ec(nc.sbuf_tensor("idx_sb", [P, idx_cols], mybir.dt.int16))
        slots_sb = ec(nc.sbuf_tensor("slots_sb", [P, nchunks], bf16))
        slots32_sb = ec(nc.sbuf_tensor("slots32_sb", [P, nchunks], f32))
        tmp_act = ec(nc.sbuf_tensor("tmp_act", [P, cfg.blk], bf16))
        cnt_sb = ec(nc.sbuf_tensor("cnt_sb", [1, ncalls], mybir.dt.int32))
        inn_sb = ec(nc.sbuf_tensor("inn_sb", [P, nblk * cfg.blk], f32))
        G = 4
        iota_sb = ec(nc.sbuf_tensor("iota_sb", [P, G, cfg.blk], bf16))
        m_sbs = [
            ec(nc.sbuf_tensor(f"m{j}", [P, cfg.cpb_max, D], bf16))
            for j in range(cfg.nb_m)
        ]
        s_sbs = [
            ec(nc.sbuf_tensor(f"s{j}", [P, G, cfg.blk], bf16))
            for j in range(cfg.nb_s)
        ]
        obufs = [ec(nc.sbuf_tensor(f"ob{j}", [P, cfg.blk], f32)) for j in range(2)]
        psums = [ec(nc.psum_tensor(f"ps{j}", [P, cfg.blk], f32)) for j in range(2)]

        io = ec(nc.semaphore("io"))
        init = ec(nc.semaphore("init"))
        gsems = [ec(nc.semaphore(f"gat{j}")) for j in range(cfg.nb_m)]
        sdve = ec(nc.semaphore("sdve"))
        sact = ec(nc.semaphore("sact"))
        pe = ec(nc.semaphore("pe"))
        ev = ec(nc.semaphore("ev"))
        osems = [ec(nc.semaphore(f"odma{j}")) for j in range(2)]

        # chunks are built in groups of G=4; group i goes to ACT iff
        # i % 4 == 3 (four single 2-pass builds), else one grouped DVE op.
        ng = cdiv(nchunks, 4)
        group_is_act = [i % 4 == 3 for i in range(ng)]

        def evict(v, bb):
            v.wait_ge(pe, cum[(bb + 1) * W])
            if bb >= 2:
                v.wait_ge(osems[bb % 2], 16 * (bb // 2))
            v.tensor_tensor(
                out=obufs[bb % 2][:],
                in0=psums[bb % 2][:],
                in1=inn_sb[:, bb * cfg.blk:(bb + 1) * cfg.blk],
                op=mybir.AluOpType.mult,
            ).then_inc(ev, 1)

        with nc.Block() as block:

            @block.sync
            def _(sync):
                sync.dma_start(idx_sb[:], idx_d[:]).then_inc(io, 16)
                sync.dma_start(slots_sb[:], slots_d[:]).then_inc(io, 16)
                sync.dma_start(slots32_sb[:], slots32_d[:]).then_inc(io, 16)
                sync.dma_start(cnt_sb[:], cnt_d[:]).then_inc(io, 16)
                sync.dma_start(inn_sb[:], inn_d[:]).then_inc(io, 16)
                for b in range(nblk):
                    sync.wait_ge(ev, b + 1)
                    sync.dma_start(
                        out_d[:, b * cfg.blk:(b + 1) * cfg.blk], obufs[b % 2][:]
                    ).then_inc(osems[b % 2], 16)
                sync.wait_ge(osems[0], 16 * cdiv(nblk, 2))
                if nblk > 1:
                    sync.wait_ge(osems[1], 16 * (nblk // 2))

            @block.gpsimd
            def _(g):
                g.iota(
                    iota_sb[:], [[0, G], [1, cfg.blk]], channel_multiplier=0,
                    allow_small_or_imprecise_dtypes=True,
                ).then_inc(init, 1)
                for j in range(cfg.nb_m):
                    g.memset(m_sbs[j][:], 0).then_inc(init, 1)
                g.load_library(mlp)
                g.wait_ge(init, 1 + cfg.nb_m)
                g.wait_ge(io, 80)
                with g.register("cnt") as cnt:
                    for k in range(ncalls):
                        w = k % W
                        if k >= cfg.nb_m:
                            # buffer k%nb_m reused: bucket k-nb_m consumed
                            g.wait_ge(pe, cum[k - cfg.nb_m + 1])
                        g.reg_load(cnt, cnt_sb[0:1, k:k + 1])
                        rows = min(cfg.win, cfg.n_nodes - w * cfg.win)
                        g.dma_gather(
                            m_sbs[k % cfg.nb_m][:, :cpbk[k], :],
                            nf[w * cfg.win: w * cfg.win + rows, :],
                            idx_sb[:, cum[k] * 8:cum[k + 1] * 8],
                            cpbk[k] * P,
                            cnt,
                            D,
                            queue_num=k % 4,
                        ).then_inc(gsems[k % cfg.nb_m], 16)

            @block.vector
            def _(v):
                v.wait_ge(io, 80)
                v.wait_ge(init, 1)
                evict_ptr = 0
                for i in range(ng):
                    start = i * G
                    end = min(nchunks, start + G)
                    gl = end - start
                    if not group_is_act[i]:
                        if i >= cfg.nb_s:
                            v.wait_ge(pe, (i - cfg.nb_s + 1) * G)
                        v.tensor_tensor(
                            out=s_sbs[i % cfg.nb_s][:, :gl, :],
                            in0=iota_sb[:, :gl, :],
                            in1=slots_sb[:, start:end]
                            .broadcast_to([P, gl, cfg.blk]),
                            op=mybir.AluOpType.is_equal,
                        ).then_inc(sdve, 1)
                    while (evict_ptr + 1 < nblk
                           and end > cum[(evict_ptr + 1) * W]):
                        evict(v, evict_ptr)
                        evict_ptr += 1
                while evict_ptr < nblk - 1:
                    evict(v, evict_ptr)
                    evict_ptr += 1
                evict(v, nblk - 1)

            @block.scalar
            def _(a):
                a.wait_ge(io, 80)
                a.wait_ge(init, 1)
                for i in range(ng):
                    if not group_is_act[i]:
                        continue
                    start = i * G
                    end = min(nchunks, start + G)
                    if i >= cfg.nb_s:
                        a.wait_ge(pe, (i - cfg.nb_s + 1) * G)
                    for t in range(start, end):
                        a.activation(
                            tmp_act[:], iota_sb[:, 0, :],
                            mybir.ActivationFunctionType.Abs,
                            bias=slots32_sb[:, t:t + 1],
                            scale=-1.0,
                        )
                        a.activation(
                            s_sbs[i % cfg.nb_s][:, t - start, :], tmp_act[:],
                            mybir.ActivationFunctionType.Relu,
                            bias=1.0,
                            scale=-1.0,
                        ).then_inc(sact, 1)

            # per-chunk producer wait targets
            dve_groups_through = []
            act_chunks_through = []
            ndg = 0
            nac = 0
            for i in range(ng):
                start = i * G
                end = min(nchunks, start + G)
                if group_is_act[i]:
                    for t in range(start, end):
                        nac += 1
                        act_chunks_through.append(nac)
                        dve_groups_through.append(None)
                else:
                    ndg += 1
                    for t in range(start, end):
                        act_chunks_through.append(None)
                        dve_groups_through.append(ndg)

            @block.tensor
            def _(te):
                t = 0
                for b in range(nblk):
                    for w in range(W):
                        k = b * W + w
                        for i in range(cpbk[k]):
                            if i == 0:
                                te.wait_ge(
                                    gsems[k % cfg.nb_m],
                                    16 * (k // cfg.nb_m + 1),
                                )
                            gi = t // G
                            if group_is_act[gi]:
                                te.wait_ge(sact, act_chunks_through[t])
                            else:
                                te.wait_ge(sdve, dve_groups_through[t])
                            start = (w == 0 and i == 0)
                            stop = (w == W - 1 and i == cpbk[k] - 1)
                            if start and b >= 2:
                                te.wait_ge(ev, b - 1)
                            te.matmul(
                                psums[b % 2][:],
                                m_sbs[k % cfg.nb_m][:, i, :],
                                s_sbs[gi % cfg.nb_s][:, t - gi * G, :],
                                start=start,
                                stop=stop,
                            ).then_inc(pe, 1)
                            t += 1

    nc.compile()
    return nc


def unshard(cfg: Cfg, results):
    out = np.empty((cfg.n_nodes, cfg.d), np.float32)
    for c in range(cfg.n_cores):
        o = results[c]["out"]
        out[c * cfg.dpc:(c + 1) * cfg.dpc, :] = o[:, :cfg.dpc].T
    return out


def run(inputs, trace=False, **spmd_kwargs):
    from concourse.bass_utils import run_bass_kernel_spmd

    cfg, in_maps = prep_host(
        inputs["u_f"], inputs["v_f"], inputs["src"], inputs["dst"]
    )
    nc = build_nc(cfg)
    res = run_bass_kernel_spmd(
        nc, in_maps, core_ids=list(range(cfg.n_cores)), trace=trace,
        **spmd_kwargs,
    )
    return unshard(cfg, res.results), res


def kernel(**inputs):
    return run(inputs)[0]
